# revision 78
# baseline (speedup 1.0000x reference)
"""Trainium2 Bass kernel for nn_ChiralEmbeddingModel — atom-major, hi/lo fp8.

Strategy (8 NeuronCores, pure data-parallel over atoms;
322931 -> 193870 -> 156145 ns in the concourse cost-model timeline sim):
 - host folds all static rescales into the weights (inv-normalization into
   g_w1/g_b1; rms_gamma, 1/sqrt(M), w_cross/w_dot and their path norms into
   W0/Wy1/Wy2; per-atom equivariant-RMS 1/rms skipped: LayerNorm cancels it)
 - host pre-transposes activations to feature-major, packs eq+inv into ONE
   fp8 input tensor laid out tile-major (each tile's DMA is one contiguous
   8KB/partition read), and packs all weights into 3 tensors (3 DMAs)
 - GEMMs in error-compensated hi/lo fp8e4m3 with DoubleRow perf mode,
   atom-major (outputs land [atoms, k] in PSUM -> no transposes anywhere)
 - engine plan (per 512-atom tile, cost-model ns):
     ACT  ~3.7us: Axy chunk0 copy (825) + silu x4 (2448) + tanh (398)
     DVE  ~4.1us: Axy chunk1 copy (925) + CR batched fp16-2x (460) + PD
                  STT x2 (1050) + chi adds x2 (388) + bn_stats x4 (508) +
                  LN-apply TS x4 (4x mode, 308) + gate-merge TT (194) +
                  batched group stats-merge/Newton (~220)
     Pool ~3.5us: P products as 4 pair-ops (pr0 split by chunk so the head
                  op only waits on the early ACT copy; 3428) + per-tile
                  gate fix tanh+1 (603)
 - 2-slot software pipeline: copies(t) -> P(t) on Pool next slot ->
   CR/PD/chi/bn(t) on DVE the slot after; Az GEMMs for tile t are emitted
   2 slots late (just before PD(t)) so Az PSUM (bufs=2) lives one slot;
   g2p shares the Az PSUM ring so the gate1 gp ring stays at 4 allocs/2
   bufs (avoids an ACT<->PE ping-pong that otherwise paces the slot)
 - LayerNorm: bn_stats per (tile, ab) [hw requires 6-elem output]; the
   even/odd-half merge (mu, var) runs as batched [128, G*4] DVE math per
   group with ONE Newton-rsqrt step off the fast-inverse-sqrt seed; group
   sizes tapered [1,1,1,1,4,8,8,6,2] so applies start early and the
   end-of-run drain stays short; applies staggered one per iteration
   (LN at the slot top, gate-merge + ACT-queue store after the P blob),
   skipped on Newton-emission slots so the two DVE batches never collide
 - sigmoid via tanh (one ACT table: silu/tanh/copy); gate fix (tanh+1)
   runs on the otherwise-idle Pool right after tanh, so the apply tail is
   one fp16-2x DVE multiply: out = (tanh+1) * ((chi-mu)*rstd/2)
 - hw-verifier constraints learned: bn_stats output must be exactly 6
   elems/partition (no multi-group), Pool runs only TT add/mult + ISA ops
   (no TensorScalarPtr), TRN2 matmul PSUM output must be fp32
"""
import os
import sys

sys.path.insert(0, '/opt/trn_rl_repo')

import numpy as np

import concourse.bass as bass
import concourse.bacc as bacc
import concourse.mybir as mybir
import concourse.tile as tile
from concourse.bass_utils import run_bass_kernel_spmd

N, INV, M, K, H = 131072, 256, 256, 64, 512
N_CORES = 8
N_CORE = N // N_CORES          # 16384 atoms per core
T = 512                        # atoms per tile
NT = N_CORE // T               # 32 tiles
G = 8                          # max tiles per LayerNorm-tail group
GROUP_SIZES = [1, 1, 1, 1, 4, 8, 8, 6, 2]
# engine placement flags (True = Pool/gpsimd, False = DVE/vector)
CR_POOL = False
CHIH_POOL = False
CHIT_POOL = False
TAIL_POOL = False
LN_POOL = False
P_DVE = 0      # how many of the 4 P sub-ops run on DVE (from the end)
NAPPLY_POLICY = 3
DRAIN_W = 4
TAIL_SPLIT = False  # gfix on Pool TT + mult on DVE TT
XIN_SPLIT = False
COPY1_SPLIT = False
P_CHUNKED = False
P_SPLIT5 = False
GFIX_POOL = True
INP_BUFS = 6
AC_BUFS = 3
P_BUFS = 4
LAG = 2                        # slots between copies(t) and the DVE chain(t)
LN_EPS = 1e-5
F16 = mybir.dt.float16
F32 = mybir.dt.float32
I32 = mybir.dt.int32
AF = mybir.ActivationFunctionType
ALU = mybir.AluOpType

LAST_RESULT = None  # BassKernelResults of the most recent run (for profiling)
_NC_CACHE = None


def _ap_view(t, offset_elems, dims):
    """Raw AP on tile t's tensor: partition dim kept, custom free dims."""
    return bass.AP(tensor=t.tensor, offset=t.offset + offset_elems,
                   ap=[list(t.ap[0])] + [list(d) for d in dims])


F8 = mybir.dt.float8e4
DR = mybir.MatmulPerfMode.DoubleRow


def build_nc():
    nc = bacc.Bacc("TRN2", target_bir_lowering=False)
    # one packed input tensor: rows 0-5 eq_hi (mh*3+c), 6-11 eq_lo,
    # 12-13 inv_hi (mh), 14-15 inv_lo
    xin = nc.dram_tensor("xin", [NT, 128, 16, T], F8, kind="ExternalInput")
    # packed weights: wp8 = [m1|m2|m3|gw1a|gw1b|gw1c] fp8, wp32 = gb1,
    # wp16 = gw2 -- one DMA each (off the SP queue) to shorten the fill
    wp8 = nc.dram_tensor("wp8", [128, 4224], F8, kind="ExternalInput")
    wp32 = nc.dram_tensor("wp32", [128, 4], F32, kind="ExternalInput")
    wp16 = nc.dram_tensor("wp16", [128, 4 * K], F16, kind="ExternalInput")
    out = nc.dram_tensor("out", [NT, 128, 4, K], F16, kind="ExternalOutput")

    with tile.TileContext(nc) as tc:
        with (
            tc.tile_pool(name="const", bufs=1) as const,
            tc.tile_pool(name="inp", bufs=INP_BUFS) as inp,
            tc.tile_pool(name="work", bufs=2) as work,
            tc.tile_pool(name="keep", bufs=G + 6) as keep,
            tc.tile_pool(name="grp", bufs=2) as grp,
            tc.tile_pool(name="ps", bufs=1, space="PSUM") as ps,
        ):
            wp8_sb = const.tile([128, 4224], F8)
            nc.sync.dma_start(out=wp8_sb, in_=wp8[:, :])
            wp32_sb = const.tile([128, 4], F32)
            nc.sync.dma_start(out=wp32_sb, in_=wp32[:, :])
            wp16_sb = const.tile([128, 4 * K], F16)
            nc.sync.dma_start(out=wp16_sb, in_=wp16[:, :])
            # weight views into the packed fp8 tile
            m1a = _ap_view(wp8_sb, 0, [[192, 2], [1, 128]])
            m1z = _ap_view(wp8_sb, 128, [[192, 2], [1, 64]])
            m2a = _ap_view(wp8_sb, 384, [[192, 2], [1, 128]])
            m2z = _ap_view(wp8_sb, 384 + 128, [[192, 2], [1, 64]])
            m3a = _ap_view(wp8_sb, 768, [[192, 2], [1, 128]])
            m3z = _ap_view(wp8_sb, 768 + 128, [[192, 2], [1, 64]])
            gw1_v = lambda which, hb: _ap_view(
                wp8_sb, 1152 + which * 1024 + hb * 128, [[H, 2], [1, 128]])
            gb1_p = lambda hb: _ap_view(wp32_sb, hb, [[1, 1]])
            gw2_v = lambda hh: _ap_view(wp16_sb, hh * K, [[1, K]])
            ones4 = const.tile([128, 4, K], F16)
            nc.vector.memset(ones4, 1.0)

            # per-tile pipeline state, indexed by tile
            in_sbs = [None] * NT
            Acs = [None] * NT
            Ps = [None] * NT
            g1ss = [None] * NT
            chiTs, tanhAs, mvG, pend, apply_q = [], [], None, None, []

            # tapered LN groups: big groups amortize the Newton batch, the
            # small final groups keep the end-of-run drain short
            tile_grp = []
            for gs in GROUP_SIZES:
                tile_grp.extend((ti, gs) for ti in range(gs))
            assert len(tile_grp) == NT

            def eq_views(sb, ab, c):
                eh = _ap_view(sb, c * T + ab * 128, [[3 * T, 2], [1, 128]])
                el = _ap_view(sb, (6 + c) * T + ab * 128,
                              [[3 * T, 2], [1, 128]])
                return eh, el

            for it in range(NT + LAG):
                u = it               # front tile: DMA, Axy, copies, gate1
                w = it - 1           # P products on Pool, gate2+tanh
                v = it - LAG         # DVE chain: CR/PD/chi/bn

                if u < NT:
                    in_sb = inp.tile([128, 16, T], F8, tag="in",
                                     name=f"in{u}")
                    nc.sync.dma_start(out=in_sb, in_=xin[u])
                    in_sbs[u] = in_sb

                # ---- staggered applies: LN (DVE) at the top of the slot so
                # its result is ready long before the Pool tail; the tail
                # itself is emitted after the P blob (below) so it never
                # delays the Pool P start. Drain 2/slot near the end.
                if NAPPLY_POLICY == 3:
                    # skip the apply on Newton-emission slots (both load DVE)
                    napply = (0 if pend is not None and it < NT - DRAIN_W else
                              2 if it >= NT - DRAIN_W and len(apply_q) > 1
                              else (1 if apply_q else 0))
                elif NAPPLY_POLICY == 0:
                    napply = (2 if it >= NT - 4 and len(apply_q) > 1 else
                              (1 if apply_q else 0))
                elif NAPPLY_POLICY == 1:
                    napply = min(len(apply_q),
                                 2 if len(apply_q) > 3 else 1)
                else:
                    napply = min(len(apply_q),
                                 2 if it % 2 == 0 else 1)
                tails = []
                for _ in range(napply):
                    item = apply_q.pop(0)
                    tails.append(emit_apply_ln(nc, work, item))

                # ---- PE: Az(v) first so PD(v) on DVE unblocks early
                if v >= 0:
                    Azs = []
                    sbv = in_sbs[v]
                    for ch in range(2):
                        Az = ps.tile([128, 3, 2, K], F32, tag="Az", bufs=2,
                                     name=f"Az{v}_{ch}")
                        for abi in range(2):
                            for c in range(3):
                                o = Az[:, c, abi, :]
                                eh, el = eq_views(sbv, ch * 2 + abi, c)
                                nc.tensor.matmul(o, eh, m1z,
                                                 perf_mode=DR, start=True,
                                                 stop=False)
                                nc.tensor.matmul(o, el, m3z,
                                                 perf_mode=DR, start=False,
                                                 stop=False)
                                nc.tensor.matmul(o, eh, m2z,
                                                 perf_mode=DR, start=False,
                                                 stop=True)
                        Azs.append(Az)

                # ---- PE: Axy(u) + copies to SBUF fp16 (ACT ch0, DVE ch1)
                if u < NT:
                    # Ac[c, ab, col]: col 0-63 x0, 64-127 y1
                    Ac = work.tile([128, 3, 4, 128], F16, tag="ac", bufs=AC_BUFS,
                                   name=f"Ac{u}")
                    Acs[u] = Ac
                    for ch in range(2):
                        Axy = ps.tile([128, 3, 2, 128], F32, tag="Axy",
                                      bufs=2, name=f"Axy{u}_{ch}")
                        for abi in range(2):
                            for c in range(3):
                                o = Axy[:, c, abi, :]
                                eh, el = eq_views(in_sb, ch * 2 + abi, c)
                                nc.tensor.matmul(o, eh, m1a,
                                                 perf_mode=DR, start=True,
                                                 stop=False)
                                nc.tensor.matmul(o, el, m3a,
                                                 perf_mode=DR, start=False,
                                                 stop=False)
                                nc.tensor.matmul(o, eh, m2a,
                                                 perf_mode=DR, start=False,
                                                 stop=True)
                        oc = Ac[:, :, 2 * ch:2 * ch + 2, :]
                        if ch == 0:
                            nc.scalar.copy(out=oc, in_=Axy)
                        elif COPY1_SPLIT:
                            # c0,c1 on DVE; c2 slice on ACT (rebalance)
                            nc.vector.tensor_copy(out=Ac[:, 0:2, 2:4, :],
                                                  in_=Axy[:, 0:2])
                            nc.scalar.copy(out=Ac[:, 2:3, 2:4, :],
                                           in_=Axy[:, 2:3])
                        else:
                            nc.vector.tensor_copy(out=oc, in_=Axy)

                # ---- DVE chain for tile v: CR -> PD -> chi -> bn.
                # Emitted first on the DVE queue: its inputs (P from Pool last
                # slot, Az just queued on PE) are ready, while copy1(u) below
                # must wait for this slot's Axy GEMMs anyway.
                if v >= 0:
                    Pv = Ps[v]
                    Ps[v] = None
                    drainv = False  # Pool assist hurts drain latency
                    CR = work.tile([128, 3, 4, K], F16, tag="cr", name=f"CR{v}")
                    cr_eng = nc.gpsimd if (CR_POOL or drainv) else nc.vector
                    cr_eng.tensor_tensor(out=CR, in0=Pv[:, :, 0],
                                         in1=Pv[:, :, 1], op=ALU.subtract)
                    # PD[c, ab, k] = (CR/256) * y2, per chunk (Az PSUM)
                    PD = work.tile([128, 3, 4, K], F16, tag="pd", name=f"PD{v}")
                    for ch in range(2):
                        sl = slice(2 * ch, 2 * ch + 2)
                        nc.vector.scalar_tensor_tensor(
                            out=PD[:, :, sl, :], in0=CR[:, :, sl, :],
                            scalar=1.0 / 256.0, in1=Azs[ch],
                            op0=ALU.mult, op1=ALU.mult)
                    # chi = PD_0 + PD_1 + PD_2 (2 batched fp16 adds)
                    chiH = work.tile([128, 4, K], F16, tag="chih",
                                     name=f"chiH{v}")
                    (nc.gpsimd if (CHIH_POOL or drainv) else
                     nc.vector).tensor_tensor(
                        out=chiH, in0=PD[:, 0], in1=PD[:, 1], op=ALU.add)
                    chiT = keep.tile([128, 4, K], F16, tag="chi",
                                     bufs=G + 6, name=f"chiT{v}")
                    (nc.gpsimd if CHIT_POOL else nc.vector).tensor_tensor(
                        out=chiT, in0=chiH, in1=PD[:, 2], op=ALU.add)
                    chiTs.append(chiT)
                    # LayerNorm stats: one multi-group bn_stats into the
                    # group stats buffer; the even/odd-half merge runs as
                    # batched [128, gsize*4] math in the group Newton phase
                    ti, gsize = tile_grp[v]
                    if ti == 0:
                        mvG = keep.tile([128, G, 4, 6], F32, tag="sg",
                                        bufs=2, name=f"sg{v}")
                    for ab in range(4):
                        nc.vector.bn_stats(out=mvG[:, ti, ab, :],
                                           in_=chiT[:, ab, :])

                # ---- PE: gate layer 1 GEMMs (u); ACT: silu
                if u < NT:
                    invh_v = _ap_view(in_sb, 12 * T, [[T, 2], [1, T]])
                    invl_v = _ap_view(in_sb, 14 * T, [[T, 2], [1, T]])
                    g1s = work.tile([128, 4, T], F16, tag="g1s", bufs=3,
                                    name=f"g1s{u}")
                    g1ss[u] = g1s
                    for hb in range(4):
                        gp = ps.tile([128, T], F32, tag="g", bufs=2,
                                     name=f"gp{u}_{hb}")
                        nc.tensor.matmul(gp, gw1_v(0, hb), invh_v,
                                         perf_mode=DR, start=True, stop=False)
                        nc.tensor.matmul(gp, gw1_v(1, hb), invl_v,
                                         perf_mode=DR, start=False, stop=False)
                        nc.tensor.matmul(gp, gw1_v(2, hb), invh_v,
                                         perf_mode=DR, start=False, stop=True)
                        nc.scalar.activation(out=g1s[:, hb, :], in_=gp,
                                             func=AF.Silu, scale=1.0 / 16.0,
                                             bias=gb1_p(hb))

                # ---- Pool: P products for tile w (3 batched pair-ops)
                # P[pr, par, ab, k]; cross_pr = P[pr,0] - P[pr,1]
                # P[pr,par] = x0_{(pr+1+par)%3} * y1_{(pr+2-par)%3}
                if 0 <= w < NT:
                    Aw = Acs[w]
                    Acs[w] = None
                    P = work.tile([128, 3, 2, 4, K], F16, tag="p", bufs=P_BUFS,
                                  name=f"P{w}")
                    Ps[w] = P
                    # (in0 offset, in0 par-stride, in1 offset, in1 par-stride)
                    specs = [
                        (512, 512, 1024 + 64, -512),      # pr=0: x0_1/2, y1_2/1
                        (1024, -1024, 64, 1024),          # pr=1: x0_2/0, y1_0/2
                        (0, 512, 512 + 64, -512),         # pr=2: x0_0/1, y1_1/0
                    ]
                    # pr=0 split by chunk so the Pool head-op only needs the
                    # early ACT copy (chunk0); pr0-ch1 tails the blob
                    def p_op(pr, ab0, nab, on_dve=False):
                        o0, s0, o1, s1 = specs[pr]
                        (nc.vector if on_dve else nc.gpsimd).tensor_tensor(
                            out=_ap_view(P, pr * 512 + 64 * ab0,
                                         [[256, 2], [64, nab], [1, K]]),
                            in0=_ap_view(Aw, o0 + 128 * ab0,
                                         [[s0, 2], [128, nab], [1, K]]),
                            in1=_ap_view(Aw, o1 + 128 * ab0,
                                         [[s1, 2], [128, nab], [1, K]]),
                            op=ALU.mult)
                    # last tiles: P on DVE (drain is latency-bound and
                    # DVE is both faster per-op and the drain-chain engine)
                    if P_SPLIT5:
                        # 3 chunk0-gated head ops, then chunk1: the Pool blob
                        # never waits on the late DVE copy for its first 2.3us
                        p_op(0, 0, 2)
                        p_op(1, 0, 2)
                        p_op(2, 0, 4)
                        p_op(0, 2, 2)
                        p_op(1, 2, 2)
                    elif P_CHUNKED:
                        # all chunk0 ops first: gated only by the early ACT
                        # copy, so the Pool blob starts ASAP
                        for pr in range(3):
                            p_op(pr, 0, 2)
                        for pr in range(3):
                            p_op(pr, 2, 2)
                    else:
                        p_op(0, 0, 2, on_dve=P_DVE >= 4)
                        p_op(1, 0, 4, on_dve=P_DVE >= 3)
                        p_op(2, 0, 4, on_dve=P_DVE >= 2)
                        p_op(0, 2, 2, on_dve=P_DVE >= 1)

                # ---- apply tails: STT + ACT store, after the P blob;
                # alternate DVE/Pool to split the tail load
                for outLN, tanhK, oidx in tails:
                    emit_apply_tail(nc, work, outLN, tanhK, oidx, out,
                                    on_pool=TAIL_POOL, ones4=ones4)

                # ---- PE: gate layer 2 (w); ACT: tanh
                if 0 <= w < NT:
                    # g2p lives in the Az PSUM ring (same 1-bank size
                    # class); emitted after Az(v) so its ring WAR (PD(v)ch0)
                    # resolves mid-slot
                    g2p = ps.tile([128, 4, K], F32, tag="Az", bufs=2,
                                  padded_shape=[128, 4, 128], name=f"g2p{w}")
                    g1w = g1ss[w]
                    g1ss[w] = None
                    for ab in range(4):
                        for hh in range(4):
                            nc.tensor.matmul(g2p[:, ab, :],
                                             g1w[:, hh, ab * 128:(ab + 1) * 128],
                                             gw2_v(hh),
                                             start=(hh == 0), stop=(hh == 3))
                    tanhA = keep.tile([128, 4, K], F16, tag="th", bufs=G + 6,
                                      name=f"tanh{w}")
                    nc.scalar.activation(out=tanhA, in_=g2p, func=AF.Tanh,
                                         scale=0.5)
                    if GFIX_POOL:
                        # gate fix (tanh+1) on the idle Pool right away; the
                        # apply tail is then a cheap fp16 2x TT on DVE
                        gfxA = keep.tile([128, 4, K], F16, tag="gfx",
                                         bufs=G + 6, name=f"gfx{w}")
                        nc.gpsimd.tensor_tensor(out=gfxA, in0=tanhA,
                                                in1=ones4, op=ALU.add)
                        tanhAs.append(gfxA)
                    else:
                        tanhAs.append(tanhA)

                # ---- group tail, software-pipelined: Newton one tile late,
                # applies staggered one per iteration (emitted at the top of
                # the next iteration)
                if v >= 0:
                    if pend is not None:
                        apply_q.extend(emit_group_newton(nc, grp, pend))
                        pend = None
                    ti, gsize = tile_grp[v]
                    if ti == gsize - 1:
                        pend = (mvG, chiTs, tanhAs[:gsize], v - gsize + 1,
                                gsize)
                        chiTs, tanhAs = [], tanhAs[gsize:]
            if pend is not None:
                apply_q.extend(emit_group_newton(nc, grp, pend))
            while apply_q:
                ln = emit_apply_ln(nc, work, apply_q.pop(0))
                emit_apply_tail(nc, work, *ln, out, ones4=ones4)
    nc.compile()
    return nc


def emit_group_newton(nc, grp, pend):
    """Batched group-stats merge + Newton rsqrt; returns apply work items.

    bn_stats 6-vec per (tile, ab) = [n_e, mean_e, n_e*var_e | n_o, mean_o,
    n_o*var_o] over even/odd elements (32 each of K=64). Merged:
      mu  = (mean_e + mean_o)/2
      var = (M2_e + M2_o)/64 + (mean_e - mean_o)^2/4
    """
    mvG, chiTs, tanhAs, t0, gsize = pend
    GA = gsize * 4
    sv = lambda off: _ap_view(mvG, off, [[6, GA]])
    muG = grp.tile([128, GA], F32, tag="mu", bufs=3)
    meanS = grp.tile([128, GA], F32, tag="nt", bufs=2)
    nc.vector.tensor_tensor(out=meanS, in0=sv(1), in1=sv(4), op=ALU.add)
    nc.vector.tensor_scalar(out=muG, in0=meanS, scalar1=0.5, scalar2=None,
                            op0=ALU.mult)
    m2S = grp.tile([128, GA], F32, tag="nt4", bufs=2)
    nc.vector.tensor_tensor(out=m2S, in0=sv(2), in1=sv(5), op=ALU.add)
    dmean = grp.tile([128, GA], F32, tag="nt5", bufs=2)
    nc.vector.tensor_tensor(out=dmean, in0=sv(1), in1=sv(4), op=ALU.subtract)
    dm2 = grp.tile([128, GA], F32, tag="nt6", bufs=2)
    nc.vector.tensor_tensor(out=dm2, in0=dmean, in1=dmean, op=ALU.mult)
    vepsA = grp.tile([128, GA], F32, tag="nt7", bufs=2)
    nc.vector.tensor_scalar(out=vepsA, in0=m2S, scalar1=1.0 / K,
                            scalar2=LN_EPS, op0=ALU.mult, op1=ALU.add)
    veps = grp.tile([128, GA], F32, tag="nt8", bufs=2)
    nc.vector.scalar_tensor_tensor(out=veps, in0=dm2, scalar=0.25,
                                   in1=vepsA, op0=ALU.mult, op1=ALU.add)
    ii = grp.tile([128, GA], I32, tag="nt", bufs=2)
    nc.vector.tensor_scalar(out=ii, in0=veps.bitcast(I32),
                            scalar1=1, scalar2=-1,
                            op0=ALU.arith_shift_right,
                            op1=ALU.bitwise_xor)
    rstd = grp.tile([128, GA], F32, tag="nt2", bufs=2)
    nc.vector.tensor_scalar(out=rstd.bitcast(I32), in0=ii,
                            scalar1=0x5f3759df + 1, scalar2=None,
                            op0=ALU.add)
    tN = grp.tile([128, GA], F32, tag="nt3", bufs=2)
    rstdh = grp.tile([128, GA], F32, tag="rs", bufs=3)
    # one Newton step (seed err ~3.4% -> ~2e-3 rel, inside the LN budget);
    # the sigmoid 0.5 folds into the final multiply
    nc.vector.tensor_tensor(out=tN, in0=rstd, in1=rstd, op=ALU.mult)
    nc.vector.tensor_tensor(out=tN, in0=tN, in1=veps, op=ALU.mult)
    nc.vector.tensor_scalar(out=tN, in0=tN, scalar1=-0.25,
                            scalar2=0.75, op0=ALU.mult, op1=ALU.add)
    nc.vector.tensor_tensor(out=rstdh, in0=rstd, in1=tN, op=ALU.mult)

    return [(muG, rstdh, chiTs[tj], tanhAs[tj], tj, t0 + tj)
            for tj in range(gsize)]


def emit_apply_ln(nc, work, item):
    """LN apply on DVE (4x-mode TS): outLN = (chi - mu) * rstd/2."""
    muG, rstdh, chiK, tanhK, tj, oidx = item
    outLN = work.tile([128, 4, K], F16, tag="oln", bufs=4)
    ln_eng = nc.gpsimd if LN_POOL else nc.vector
    for ab in range(4):
        ln_eng.tensor_scalar(
            out=outLN[:, ab, :], in0=chiK[:, ab, :],
            scalar1=muG[:, tj * 4 + ab:tj * 4 + ab + 1],
            scalar2=rstdh[:, tj * 4 + ab:tj * 4 + ab + 1],
            op0=ALU.subtract, op1=ALU.mult)
    return outLN, tanhK, oidx


def emit_apply_tail(nc, work, outLN, tanhK, oidx, out, on_pool=False,
                    ones4=None):
    """out = (tanh + 1) * outLN: one DVE STT, or two Pool TTs (the Pool
    engine only supports TensorTensor add/mult)."""
    outF = work.tile([128, 4, K], F16, tag="of", bufs=4)
    if on_pool:
        gfix = work.tile([128, 4, K], F16, tag="gf", bufs=2)
        nc.gpsimd.tensor_tensor(out=gfix, in0=tanhK, in1=ones4, op=ALU.add)
        nc.gpsimd.tensor_tensor(out=outF, in0=gfix, in1=outLN, op=ALU.mult)
    elif TAIL_SPLIT:
        gfix = work.tile([128, 4, K], F16, tag="gf", bufs=2)
        nc.gpsimd.tensor_tensor(out=gfix, in0=tanhK, in1=ones4, op=ALU.add)
        nc.vector.tensor_tensor(out=outF, in0=gfix, in1=outLN, op=ALU.mult)
    elif GFIX_POOL:
        nc.vector.tensor_tensor(out=outF, in0=tanhK, in1=outLN, op=ALU.mult)
    else:
        nc.vector.scalar_tensor_tensor(out=outF, in0=tanhK, scalar=1.0,
                                       in1=outLN, op0=ALU.add, op1=ALU.mult)
    # stores go out the ACT queue (HWDGE frees SEQ after descriptor gen)
    nc.scalar.dma_start(out=out[oidx], in_=outF)


try:
    import ml_dtypes
    _F8NP = ml_dtypes.float8_e4m3
except ImportError:  # pragma: no cover
    _F8NP = None


def _hilo8(x):
    """Split x (float) into fp8 hi + fp8 lo where x ~ hi + lo16/16."""
    hi = x.astype(_F8NP)
    lo16 = ((x - hi.astype(np.float64)) * 16.0).astype(_F8NP)
    return hi, lo16


def _prep_weights(mean_inv, std_inv, rms_gamma, W0, W1, W2, w_cross, w_dot,
                  g_w1, g_b1, g_w2):
    g = (rms_gamma.astype(np.float64) / np.sqrt(M))
    W0s = W0.astype(np.float64) * g[:, None]
    Wy1 = (W1.astype(np.float64) * g[:, None]) @ (w_cross.T.astype(np.float64) / np.sqrt(2.0 * K))
    Wy2 = (W2.astype(np.float64) * g[:, None]) @ (w_dot.T.astype(np.float64) / np.sqrt(3.0 * K))
    wall = np.concatenate([W0s, Wy1, Wy2], axis=1)          # [256, 192]
    wall_r = wall.reshape(2, 128, 192).transpose(1, 0, 2) * 16.0
    m1h, m1lo = _hilo8(wall_r)
    m2 = (m1lo.astype(np.float32) / 16.0).astype(_F8NP)
    m3 = (m1h.astype(np.float32) / 16.0).astype(_F8NP)
    inv_std = 1.0 / std_inv.astype(np.float64)
    GW1 = g_w1.astype(np.float64) * inv_std[:, None]
    gw1_r = GW1.reshape(2, 128, H).transpose(1, 0, 2) * 16.0
    g1h, g1lo = _hilo8(gw1_r)
    gw1b = (g1h.astype(np.float32) / 16.0).astype(_F8NP)
    gw1c = (g1lo.astype(np.float32) / 16.0).astype(_F8NP)
    GB1 = g_b1.astype(np.float64) - (mean_inv.astype(np.float64) * inv_std) @ g_w1.astype(np.float64)
    gb1_r = GB1.reshape(4, 128).T.astype(np.float32).copy()
    gw2_r = g_w2.astype(np.float64).reshape(4, 128, K).transpose(1, 0, 2).astype(np.float16)
    wp8 = np.concatenate(
        [m1h.reshape(128, -1), m2.reshape(128, -1), m3.reshape(128, -1),
         g1h.reshape(128, -1), gw1b.reshape(128, -1), gw1c.reshape(128, -1)],
        axis=1)
    return dict(wp8=np.ascontiguousarray(wp8), wp32=gb1_r,
                wp16=np.ascontiguousarray(gw2_r.reshape(128, -1)))


def kernel(atomic_embeddings, mean_inv, std_inv, rms_gamma, W0, W1, W2,
           w_cross, w_dot, ln_w, ln_b, g_w1, g_b1, g_w2, g_b2):
    global _NC_CACHE, LAST_RESULT
    assert np.allclose(np.asarray(ln_w), 1.0) and np.allclose(np.asarray(ln_b), 0.0), \
        "kernel specialized for ln_w=1, ln_b=0"
    assert np.allclose(np.asarray(g_b2), 0.0), "kernel specialized for g_b2=0"
    weights = _prep_weights(np.asarray(mean_inv), np.asarray(std_inv),
                            np.asarray(rms_gamma), np.asarray(W0), np.asarray(W1),
                            np.asarray(W2), np.asarray(w_cross), np.asarray(w_dot),
                            np.asarray(g_w1), np.asarray(g_b1), np.asarray(g_w2))
    emb = np.asarray(atomic_embeddings)
    if _NC_CACHE is None:
        _NC_CACHE = build_nc()
    nc = _NC_CACHE
    in_maps = []
    for cc in range(N_CORES):
        ec = emb[cc * N_CORE:(cc + 1) * N_CORE]
        inv = ec[:, :INV]
        eq = ec[:, INV:].reshape(N_CORE, M, 3)
        # eqT[p, mh, c, n] = eq[n, mh*128+p, c]
        eqT = np.ascontiguousarray(
            eq.transpose(1, 2, 0).reshape(2, 128, 3, N_CORE).transpose(1, 0, 2, 3))
        invT = np.ascontiguousarray(
            inv.T.reshape(2, 128, N_CORE).transpose(1, 0, 2))
        eh, el = _hilo8(eqT)
        ih, il = _hilo8(invT)
        xin = np.empty((128, 16, N_CORE), dtype=_F8NP)
        xin[:, 0:6] = eh.reshape(128, 6, N_CORE)
        xin[:, 6:12] = el.reshape(128, 6, N_CORE)
        xin[:, 12:14] = ih
        xin[:, 14:16] = il
        # tile-major so each tile's DMA is one contiguous 8KB/partition read
        xin = np.ascontiguousarray(
            xin.reshape(128, 16, NT, T).transpose(2, 0, 1, 3))
        m = dict(weights)
        m["xin"] = xin
        in_maps.append(m)
    trace = bool(int(os.environ.get("CHIRAL_TRACE", "0")))
    try:
        from antenv import axon_hooks  # noqa: F401
    except ImportError:
        # NTFF profiling hook absent in this container: tracing would crash
        # inside run_bass_kernel_spmd, so force it off.
        os.environ["BASS_NEVER_TRACE"] = "1"
        trace = False
    res = run_bass_kernel_spmd(nc, in_maps, core_ids=list(range(N_CORES)),
                               trace=trace)
    LAST_RESULT = res
    outs = []
    for cc in range(N_CORES):
        o = res.results[cc]["out"]               # [NT, 128, 4, K] fp16
        outs.append(o.transpose(0, 2, 1, 3).reshape(N_CORE, K).astype(np.float32))
    return np.concatenate(outs, axis=0)
